# revision 16
# baseline (speedup 1.0000x reference)
"""DepthConv kernel for Trainium2 (Bass/Tile), data-parallel over batch on 8 cores.

Problem: out[b,o,x,y] = sum_{c,k} w[o,c,k] * data[b,c,x+i,y+j] * aff[b,k,x,y]
         aff[b,k,x,y] = exp(-8.3*|depth[b,x+i,y+j] - depth[b,x+1,y+1]|), k=(i,j) in 3x3
Shapes: data [8,16,256,256], depth [8,1,256,256], weight [16,16,3,3] -> out [8,16,254,254]

Per-core layout (1 image/core): partitions = (strip q=0..7, channel c=0..15).
Each strip covers 32 output rows; free dim n = xl*256+y (flat, row-wrapping).
 - 3x3 taps become pure free-dim shifts (i*256+j) of one resident data tile.
 - Per-tap matmul uses block-diagonal weights [(q,c),(q,o)] so all 8 strips'
   channel contractions run in one full-width 128x128 matmul; 9 taps
   PSUM-accumulate.
 - Main loop runs at 512-px (2 output rows/strip) granularity and computes the
   affinity inline (center-select matmul + ACT abs/exp), immediately followed
   by 8 tap-broadcast matmuls, DVE/ACT multiplies, and 9 accumulating output
   matmuls; the output tile goes PSUM -> DRAM in a single strided DMA, so no
   SBUF output staging and no serial affinity prologue.
 - Tap multiplies are balanced across engines: 3 taps multiply straight from
   PSUM on the DVE, 5 taps get an ACT f16 copy first so the DVE runs in 2x
   mode; ACT also does the abs/exp.
"""

import numpy as np

B, C, H, W = 8, 16, 256, 256
O, KH, KW = 16, 3, 3
ALPHA = 8.3
OH, OW = H - KH + 1, W - KW + 1  # 254, 254
P = 128
NQ, QROWS = 8, 32           # strips, output rows per strip
NFREE = QROWS * W           # 8192 flat pixels per strip (incl. y>=254 garbage)
NTILE = 512                 # 2 output rows per tile
NT = NFREE // NTILE         # 16 n-tiles
DWIN = 34 * W + 16          # data window: 34 rows halo + shift pad
TAPS = [(i, j) for i in range(KH) for j in range(KW)]
NC_KS = [k for k in range(9) if k != 4]  # non-center taps
NBLK = 18                   # 9 weight blocks + 8 tap-select + 1 center-select
D0 = 0                      # data window offset in the packed tensor
Z0 = DWIN                   # dep_t offset
M0 = DWIN + NFREE           # wsmat offset
TOT = DWIN + NFREE + NBLK * P
DIRECT_KS = (0, 2, 5, 7)    # taps multiplied straight from PSUM on DVE

_CACHE = {}


def _build_nc():
    import concourse.bass as bass
    import concourse.bacc as bacc
    import concourse.mybir as mybir
    from concourse.tile import TileContext
    from concourse.alu_op_type import AluOpType
    from concourse.bass_types import AP

    f32 = mybir.dt.float32
    f16 = mybir.dt.float16
    AF = mybir.ActivationFunctionType

    nc = bacc.Bacc(None, target_bir_lowering=False)
    allin_d = nc.dram_tensor("allin", [P, TOT], f16, kind="ExternalInput")
    out_d = nc.dram_tensor("out", [O, OH, OW], f16, kind="ExternalOutput")

    with TileContext(nc) as tc:
        with (
            tc.tile_pool(name="const", bufs=1) as cpool,
            tc.tile_pool(name="apool", bufs=3) as apool,
            tc.tile_pool(name="vpool", bufs=6) as vpool,
            tc.tile_pool(name="spool", bufs=4) as spool,
            tc.tile_pool(name="zcps", bufs=2, space="PSUM") as zcps,
            tc.tile_pool(name="affps", bufs=3, space="PSUM") as affps,
            tc.tile_pool(name="outps", bufs=3, space="PSUM") as outps,
        ):
            allin = cpool.tile([P, TOT], f16)
            # chunked load: selection+weight matrices first, then interleaved
            # depth/data quarters so the first tiles' compute overlaps the
            # remaining transfers
            m17 = M0 + 17 * P
            nc.sync.dma_start(allin[:, m17 : m17 + P], allin_d[:, m17 : m17 + P])
            nc.sync.dma_start(allin[:, M0:m17], allin_d[:, M0:m17])
            nq4 = 8
            dq = (DWIN + nq4 - 1) // nq4
            zq = NFREE // nq4
            for cch in range(nq4):
                za, zb = Z0 + cch * zq, Z0 + (cch + 1) * zq
                nc.sync.dma_start(allin[:, za:zb], allin_d[:, za:zb])
                a, bnd = cch * dq, min(DWIN, (cch + 1) * dq)
                nc.sync.dma_start(allin[:, a:bnd], allin_d[:, a:bnd])

            def seg(off, size):
                return allin[:, off : off + size]

            for u in range(NT):
                base = u * NTILE
                # inline affinity: center-diff matmul + Pool abs + ACT exp
                zc = zcps.tile([P, NTILE], f32, tag="zc")
                nc.tensor.matmul(
                    zc[:], seg(M0 + 17 * P, P), seg(Z0 + base, NTILE),
                    start=True, stop=True,
                )
                afft = apool.tile([P, NTILE], f16, tag="afft")
                nc.scalar.activation(afft[:], zc[:], AF.Abs, scale=-ALPHA)
                nc.scalar.activation(afft[:], afft[:], AF.Exp, scale=-1.0)

                outp = outps.tile([P, NTILE], f32, tag="outp")
                taps = [1, 3, 6, 8, 0, 2, 5, 7]  # ACT-copy taps first
                ap2s = {}

                def bcast(k):
                    jj = NC_KS.index(k)
                    ap2 = affps.tile([P, NTILE], f32, tag="ap2")
                    nc.tensor.matmul(
                        ap2[:], seg(M0 + (9 + jj) * P, P), afft[:],
                        start=True, stop=True, skip_group_check=True,
                    )
                    ap2s[k] = ap2

                # PE lookahead: 2 broadcasts in flight before each output
                bcast(taps[0])
                bcast(taps[1])
                # center tap opens the accumulation (needs no multiply)
                nc.tensor.matmul(
                    outp[:], seg(M0 + 4 * P, P),
                    seg(D0 + base + 1 * W + 1, NTILE),
                    start=True, stop=False, skip_group_check=True,
                )
                for idx, k in enumerate(taps):
                    if idx + 2 < len(taps):
                        bcast(taps[idx + 2])
                    i, j = TAPS[k]
                    shift = base + i * W + j
                    ap2 = ap2s.pop(k)
                    v2 = vpool.tile([P, NTILE], f16, tag="v")
                    if k in DIRECT_KS:
                        nc.vector.tensor_tensor(
                            v2[:], seg(D0 + shift, NTILE), ap2[:],
                            AluOpType.mult,
                        )
                    else:
                        ap_sb = spool.tile([P, NTILE], f16, tag="apsb")
                        nc.scalar.copy(ap_sb[:], ap2[:])
                        eng = nc.vector if k == 1 else nc.gpsimd
                        eng.tensor_tensor(
                            v2[:], seg(D0 + shift, NTILE), ap_sb[:],
                            AluOpType.mult,
                        )
                    nc.tensor.matmul(
                        outp[:], seg(M0 + k * P, P), v2[:],
                        start=False, stop=(idx == len(taps) - 1),
                        skip_group_check=True,
                    )

                # DVE copy PSUM -> packed f16 SBUF (2 rows x 254 contiguous),
                # then one strided DMA (partitions (q,o) -> dst dims)
                osb = spool.tile([P, 2 * OW], f16, tag="osb")
                nc.vector.tensor_copy(
                    osb[:],
                    outp[:].rearrange("p (x y) -> p x y", y=W)[:, :, 0:OW],
                )
                x0 = 2 * u
                nq_full = NQ if x0 + 2 <= 30 else NQ - 1
                dst_ap = AP(
                    out_d[:].tensor,
                    x0 * OW,
                    [[QROWS * OW, nq_full], [OH * OW, O], [1, 2 * OW]],
                )
                nc.sync.dma_start(dst_ap, osb[0 : 16 * nq_full, :])
                if nq_full < NQ:
                    # strip 7 has only 30 valid output rows (224..253)
                    nrows = max(0, min(x0 + 2, OH - QROWS * 7) - x0)
                    if nrows:
                        nc.sync.dma_start(
                            out_d[:, QROWS * 7 + x0 : QROWS * 7 + x0 + nrows, :],
                            osb[16 * 7 : 16 * 7 + 16, 0 : nrows * OW],
                        )
    nc.compile()
    return nc


def _pack_inputs(data, depth, weight):
    """Build the [B, 128, TOT] packed input: data windows, shifted depth
    taps, and the weight/selection matrices."""
    HP = H + 3
    data_p = np.zeros((B, C, HP * W), np.float32)
    data_p[:, :, : H * W] = data.reshape(B, C, H * W)
    depth_p = np.zeros((B, HP * W), np.float32)
    depth_p[:, : H * W] = depth.reshape(B, H * W)

    wsmat = np.zeros((NBLK, P, P), np.float32)
    for k in range(9):
        i, j = TAPS[k]
        blk = weight[:, :, i, j].T  # [c, o]
        for q in range(NQ):
            wsmat[k, 16 * q : 16 * q + 16, 16 * q : 16 * q + 16] = blk
    for jj, k in enumerate(NC_KS):
        for q in range(NQ):
            wsmat[9 + jj, 16 * q + k, 16 * q : 16 * q + 16] = 1.0
    wsmat[17] = np.eye(P, dtype=np.float32)
    for q in range(NQ):
        wsmat[17, 16 * q + 4, 16 * q : 16 * q + 16] -= 1.0
    wsmat_flat = wsmat.transpose(1, 0, 2).reshape(P, NBLK * P)

    allin = np.zeros((B, P, TOT), np.float16)
    for q in range(NQ):
        for c in range(C):
            p = 16 * q + c
            s = 32 * q * W
            allin[:, p, D0 : D0 + DWIN] = data_p[:, c, s : s + DWIN]
        for k, (i, j) in enumerate(TAPS):
            p = 16 * q + k
            s = (32 * q + i) * W + j
            allin[:, p, Z0 : Z0 + NFREE] = depth_p[:, s : s + NFREE]
    allin[:, :, M0:] = wsmat_flat[None]
    return allin


def run(inputs, **spmd_kwargs):
    from concourse.bass_utils import run_bass_kernel_spmd

    data = np.asarray(inputs["data"], np.float32)
    depth = np.asarray(inputs["depth"], np.float32)
    weight = np.asarray(inputs["weight"], np.float32)
    allin = _pack_inputs(data, depth, weight)

    if "nc" not in _CACHE:
        _CACHE["nc"] = _build_nc()
    nc = _CACHE["nc"]

    in_maps = [{"allin": np.ascontiguousarray(allin[b])} for b in range(B)]
    res = run_bass_kernel_spmd(nc, in_maps, core_ids=list(range(B)), **spmd_kwargs)
    out = np.stack([res.results[b]["out"] for b in range(B)]).astype(np.float32)
    return np.ascontiguousarray(out), res


def kernel(**inputs):
    out, _ = run(inputs)
    return out


# revision 18
# speedup vs baseline: 1.3102x; 1.3102x over previous
"""DepthConv kernel for Trainium2 (Bass/Tile), data-parallel over batch on 8 cores.

Problem: out[b,o,x,y] = sum_{c,k} w[o,c,k] * data[b,c,x+i,y+j] * aff[b,k,x,y]
         aff[b,k,x,y] = exp(-8.3*|depth[b,x+i,y+j] - depth[b,x+1,y+1]|), k=(i,j) in 3x3
Shapes: data [8,16,256,256], depth [8,1,256,256], weight [16,16,3,3] -> out [8,16,254,254]

Per-core layout (1 image/core): partitions = (strip q=0..7, channel c=0..15).
Each strip covers 32 output rows; free dim n = xl*256+y (flat, row-wrapping).
 - 3x3 taps become pure free-dim shifts (i*256+j) of one resident data tile.
 - Per-tap matmul uses block-diagonal weights [(q,c),(q,o)] so all 8 strips'
   channel contractions run in one full-width 128x128 matmul; 9 taps
   PSUM-accumulate.
 - Main loop runs at 512-px (2 output rows/strip) granularity and computes the
   affinity inline (center-select matmul + ACT abs/exp), immediately followed
   by 8 tap-broadcast matmuls, DVE/ACT multiplies, and 9 accumulating output
   matmuls; the output tile goes PSUM -> DRAM in a single strided DMA, so no
   SBUF output staging and no serial affinity prologue.
 - Tap multiplies are balanced across engines: 3 taps multiply straight from
   PSUM on the DVE, 5 taps get an ACT f16 copy first so the DVE runs in 2x
   mode; ACT also does the abs/exp.
"""

import numpy as np

B, C, H, W = 8, 16, 256, 256
O, KH, KW = 16, 3, 3
ALPHA = 8.3
OH, OW = H - KH + 1, W - KW + 1  # 254, 254
P = 128
NQ, QROWS = 8, 32           # strips, output rows per strip
NFREE = QROWS * W           # 8192 flat pixels per strip (incl. y>=254 garbage)
NTILE = 512                 # 2 output rows per tile
NT = NFREE // NTILE         # 16 n-tiles
DWIN = 34 * W + 16          # data window: 34 rows halo + shift pad
TAPS = [(i, j) for i in range(KH) for j in range(KW)]
NC_KS = [k for k in range(9) if k != 4]  # non-center taps
NBLK = 18                   # 9 weight blocks + 8 tap-select + 1 center-select
D0 = 0                      # data window offset in the packed tensor
Z0 = DWIN                   # dep_t offset
M0 = DWIN + NFREE           # wsmat offset
TOT = DWIN + NFREE + NBLK * P
DIRECT_KS = (0, 2, 5, 7)    # taps multiplied straight from PSUM on DVE

_CACHE = {}


def _build_nc():
    import concourse.bass as bass
    import concourse.bacc as bacc
    import concourse.mybir as mybir
    from concourse.tile import TileContext
    from concourse.alu_op_type import AluOpType
    from concourse.bass_types import AP

    f32 = mybir.dt.float32
    f16 = mybir.dt.float16
    AF = mybir.ActivationFunctionType

    nc = bacc.Bacc(None, target_bir_lowering=False)
    allin_d = nc.dram_tensor("allin", [P, TOT], f16, kind="ExternalInput")
    out_d = nc.dram_tensor("out", [O, OH, OW], f16, kind="ExternalOutput")

    with TileContext(nc) as tc:
        with (
            tc.tile_pool(name="const", bufs=1) as cpool,
            tc.tile_pool(name="apool", bufs=4) as apool,
            tc.tile_pool(name="vpool", bufs=8) as vpool,
            tc.tile_pool(name="aspool", bufs=8) as aspool,
            tc.tile_pool(name="ospool", bufs=4) as ospool,
            tc.tile_pool(name="zcps", bufs=2, space="PSUM") as zcps,
            tc.tile_pool(name="affps", bufs=4, space="PSUM") as affps,
            tc.tile_pool(name="outps", bufs=2, space="PSUM") as outps,
        ):
            allin = cpool.tile([P, TOT], f16)
            # chunked load: selection+weight matrices first, then interleaved
            # depth/data quarters so the first tiles' compute overlaps the
            # remaining transfers
            m17 = M0 + 17 * P
            nc.sync.dma_start(allin[:, m17 : m17 + P], allin_d[:, m17 : m17 + P])
            nc.sync.dma_start(allin[:, M0:m17], allin_d[:, M0:m17])
            nq4 = 8
            dq = (DWIN + nq4 - 1) // nq4
            zq = NFREE // nq4
            for cch in range(nq4):
                za, zb = Z0 + cch * zq, Z0 + (cch + 1) * zq
                nc.sync.dma_start(allin[:, za:zb], allin_d[:, za:zb])
                a, bnd = cch * dq, min(DWIN, (cch + 1) * dq)
                nc.sync.dma_start(allin[:, a:bnd], allin_d[:, a:bnd])

            def seg(off, size):
                return allin[:, off : off + size]

            # software pipeline: the affinity for tile u+1 (center matmul +
            # ACT abs/exp) is issued during tile u, so tile u's broadcasts
            # never wait on the affinity chain.
            def mk_aff(un):
                basez = un * NTILE
                zc = zcps.tile([P, NTILE], f32, tag="zc")
                nc.tensor.matmul(
                    zc[:], seg(M0 + 17 * P, P), seg(Z0 + basez, NTILE),
                    start=True, stop=True, skip_group_check=True,
                )
                afft = apool.tile([P, NTILE], f16, tag="afft")
                nc.scalar.activation(afft[:], zc[:], AF.Abs, scale=-ALPHA)
                nc.scalar.activation(afft[:], afft[:], AF.Exp, scale=-1.0)
                return afft

            # tap roles: DVE straight from PSUM; pool/dve-sbuf via ACT copy
            POOL_KS = (1, 3, 6)
            # emit order: pool-chain taps first (longest latency), then direct
            TAP_EMIT = [1, 3, 6, 8, 0, 2, 5, 7]

            affts = {0: mk_aff(0)}
            for u in range(NT):
                base = u * NTILE
                afft = affts.pop(u)
                outp = outps.tile([P, NTILE], f32, tag="outp")
                ap2s = {}

                def bcast(k, afft=afft):
                    jj = NC_KS.index(k)
                    ap2 = affps.tile([P, NTILE], f32, tag="ap2")
                    nc.tensor.matmul(
                        ap2[:], seg(M0 + (9 + jj) * P, P), afft[:],
                        start=True, stop=True, skip_group_check=True,
                    )
                    ap2s[k] = ap2

                def mult(k):
                    i, j = TAPS[k]
                    shift = base + i * W + j
                    ap2 = ap2s.pop(k)
                    v2 = vpool.tile([P, NTILE], f16, tag="v")
                    if k in DIRECT_KS:
                        nc.vector.tensor_tensor(
                            v2[:], seg(D0 + shift, NTILE), ap2[:],
                            AluOpType.mult,
                        )
                    else:
                        ap_sb = aspool.tile([P, NTILE], f16, tag="apsb")
                        nc.scalar.copy(ap_sb[:], ap2[:])
                        eng = nc.gpsimd if k in POOL_KS else nc.vector
                        eng.tensor_tensor(
                            v2[:], seg(D0 + shift, NTILE), ap_sb[:],
                            AluOpType.mult,
                        )
                    return v2

                def out_mm(k, rhs, start=False, stop=False):
                    nc.tensor.matmul(
                        outp[:], seg(M0 + k * P, P), rhs,
                        start=start, stop=stop, skip_group_check=True,
                    )

                # next tile's affinity goes first in every engine queue
                if u + 1 < NT:
                    affts[u + 1] = mk_aff(u + 1)

                bcast(TAP_EMIT[0])
                bcast(TAP_EMIT[1])
                # center tap opens the accumulation (needs no multiply)
                out_mm(4, seg(D0 + base + 1 * W + 1, NTILE), start=True)
                v2s = {}
                # phase 1: issue broadcasts + multiplies (2-ahead lookahead);
                # direct-tap outputs drain early, pool-tap outputs at the end
                for idx, k in enumerate(TAP_EMIT):
                    if idx + 2 < len(TAP_EMIT):
                        bcast(TAP_EMIT[idx + 2])
                    v2s[k] = mult(k)
                    if k not in POOL_KS:
                        out_mm(k, v2s.pop(k)[:])
                for ki, k in enumerate(POOL_KS):
                    out_mm(k, v2s.pop(k)[:], stop=(ki == len(POOL_KS) - 1))

                # DVE copy PSUM -> packed f16 SBUF (2 rows x 254 contiguous),
                # then one strided DMA (partitions (q,o) -> dst dims)
                osb = ospool.tile([P, 2 * OW], f16, tag="osb")
                nc.vector.tensor_copy(
                    osb[:],
                    outp[:].rearrange("p (x y) -> p x y", y=W)[:, :, 0:OW],
                )
                x0 = 2 * u
                nq_full = NQ if x0 + 2 <= 30 else NQ - 1
                dst_ap = AP(
                    out_d[:].tensor,
                    x0 * OW,
                    [[QROWS * OW, nq_full], [OH * OW, O], [1, 2 * OW]],
                )
                nc.sync.dma_start(dst_ap, osb[0 : 16 * nq_full, :])
                if nq_full < NQ:
                    # strip 7 has only 30 valid output rows (224..253)
                    nrows = max(0, min(x0 + 2, OH - QROWS * 7) - x0)
                    if nrows:
                        nc.sync.dma_start(
                            out_d[:, QROWS * 7 + x0 : QROWS * 7 + x0 + nrows, :],
                            osb[16 * 7 : 16 * 7 + 16, 0 : nrows * OW],
                        )
    nc.compile()
    return nc


def _pack_inputs(data, depth, weight):
    """Build the [B, 128, TOT] packed input: data windows, shifted depth
    taps, and the weight/selection matrices."""
    HP = H + 3
    data_p = np.zeros((B, C, HP * W), np.float32)
    data_p[:, :, : H * W] = data.reshape(B, C, H * W)
    depth_p = np.zeros((B, HP * W), np.float32)
    depth_p[:, : H * W] = depth.reshape(B, H * W)

    wsmat = np.zeros((NBLK, P, P), np.float32)
    for k in range(9):
        i, j = TAPS[k]
        blk = weight[:, :, i, j].T  # [c, o]
        for q in range(NQ):
            wsmat[k, 16 * q : 16 * q + 16, 16 * q : 16 * q + 16] = blk
    for jj, k in enumerate(NC_KS):
        for q in range(NQ):
            wsmat[9 + jj, 16 * q + k, 16 * q : 16 * q + 16] = 1.0
    wsmat[17] = np.eye(P, dtype=np.float32)
    for q in range(NQ):
        wsmat[17, 16 * q + 4, 16 * q : 16 * q + 16] -= 1.0
    wsmat_flat = wsmat.transpose(1, 0, 2).reshape(P, NBLK * P)

    allin = np.zeros((B, P, TOT), np.float16)
    for q in range(NQ):
        for c in range(C):
            p = 16 * q + c
            s = 32 * q * W
            allin[:, p, D0 : D0 + DWIN] = data_p[:, c, s : s + DWIN]
        for k, (i, j) in enumerate(TAPS):
            p = 16 * q + k
            s = (32 * q + i) * W + j
            allin[:, p, Z0 : Z0 + NFREE] = depth_p[:, s : s + NFREE]
    allin[:, :, M0:] = wsmat_flat[None]
    return allin


def run(inputs, **spmd_kwargs):
    from concourse.bass_utils import run_bass_kernel_spmd

    data = np.asarray(inputs["data"], np.float32)
    depth = np.asarray(inputs["depth"], np.float32)
    weight = np.asarray(inputs["weight"], np.float32)
    allin = _pack_inputs(data, depth, weight)

    if "nc" not in _CACHE:
        _CACHE["nc"] = _build_nc()
    nc = _CACHE["nc"]

    in_maps = [{"allin": np.ascontiguousarray(allin[b])} for b in range(B)]
    res = run_bass_kernel_spmd(nc, in_maps, core_ids=list(range(B)), **spmd_kwargs)
    out = np.stack([res.results[b]["out"] for b in range(B)]).astype(np.float32)
    return np.ascontiguousarray(out), res


def kernel(**inputs):
    out, _ = run(inputs)
    return out


# revision 20
# speedup vs baseline: 1.4023x; 1.0703x over previous
"""DepthConv kernel for Trainium2 (Bass/Tile), data-parallel over batch on 8 cores.

Problem: out[b,o,x,y] = sum_{c,k} w[o,c,k] * data[b,c,x+i,y+j] * aff[b,k,x,y]
         aff[b,k,x,y] = exp(-8.3*|depth[b,x+i,y+j] - depth[b,x+1,y+1]|), k=(i,j) in 3x3
Shapes: data [8,16,256,256], depth [8,1,256,256], weight [16,16,3,3] -> out [8,16,254,254]

Per-core layout (1 image/core): partitions = (strip q=0..7, channel c=0..15).
Each strip covers 32 output rows; free dim n = xl*256+y (flat, row-wrapping).
 - 3x3 taps become pure free-dim shifts (i*256+j) of one resident data tile.
 - Per-tap matmul uses block-diagonal weights [(q,c),(q,o)] so all 8 strips'
   channel contractions run in one full-width 128x128 matmul; 9 taps
   PSUM-accumulate.
 - Main loop runs at 512-px (2 output rows/strip) granularity and computes the
   affinity inline (center-select matmul + ACT abs/exp), immediately followed
   by 8 tap-broadcast matmuls, DVE/ACT multiplies, and 9 accumulating output
   matmuls; the output tile goes PSUM -> DRAM in a single strided DMA, so no
   SBUF output staging and no serial affinity prologue.
 - Tap multiplies are balanced across engines: 3 taps multiply straight from
   PSUM on the DVE, 5 taps get an ACT f16 copy first so the DVE runs in 2x
   mode; ACT also does the abs/exp.
"""

import numpy as np

B, C, H, W = 8, 16, 256, 256
O, KH, KW = 16, 3, 3
ALPHA = 8.3
OH, OW = H - KH + 1, W - KW + 1  # 254, 254
P = 128
NQ, QROWS = 8, 32           # strips, output rows per strip
NFREE = QROWS * W           # 8192 flat pixels per strip (incl. y>=254 garbage)
NTILE = 512                 # 2 output rows per tile
NT = NFREE // NTILE         # 16 n-tiles
DWIN = 34 * W + 16          # data window: 34 rows halo + shift pad
TAPS = [(i, j) for i in range(KH) for j in range(KW)]
NC_KS = [k for k in range(9) if k != 4]  # non-center taps
NBLK = 18                   # 9 weight blocks + 8 tap-select + 1 center-select
D0 = 0                      # data window offset in the packed tensor
Z0 = DWIN                   # dep_t offset
M0 = DWIN + NFREE           # wsmat offset
TOT = DWIN + NFREE + NBLK * P
DIRECT_KS = (0, 2, 5, 7)    # taps multiplied straight from PSUM on DVE

_CACHE = {}


def _build_nc():
    import concourse.bass as bass
    import concourse.bacc as bacc
    import concourse.mybir as mybir
    from concourse.tile import TileContext
    from concourse.alu_op_type import AluOpType
    from concourse.bass_types import AP

    f32 = mybir.dt.float32
    f16 = mybir.dt.float16
    AF = mybir.ActivationFunctionType

    nc = bacc.Bacc(None, target_bir_lowering=False)
    allin_d = nc.dram_tensor("allin", [P, TOT], f16, kind="ExternalInput")
    out_d = nc.dram_tensor("out", [O, OH, OW], f16, kind="ExternalOutput")

    with TileContext(nc) as tc:
        with (
            tc.tile_pool(name="const", bufs=1) as cpool,
            tc.tile_pool(name="apool", bufs=4) as apool,
            tc.tile_pool(name="vpool", bufs=8) as vpool,
            tc.tile_pool(name="aspool", bufs=8) as aspool,
            tc.tile_pool(name="ospool", bufs=4) as ospool,
            tc.tile_pool(name="zcps", bufs=1, space="PSUM") as zcps,
            tc.tile_pool(name="affps", bufs=5, space="PSUM") as affps,
            tc.tile_pool(name="outps", bufs=2, space="PSUM") as outps,
        ):
            allin = cpool.tile([P, TOT], f16)
            # chunked load: selection+weight matrices first, then interleaved
            # depth/data quarters so the first tiles' compute overlaps the
            # remaining transfers
            m17 = M0 + 17 * P
            nc.sync.dma_start(allin[:, m17 : m17 + P], allin_d[:, m17 : m17 + P])
            nc.sync.dma_start(allin[:, M0:m17], allin_d[:, M0:m17])
            nq4 = 8
            dq = (DWIN + nq4 - 1) // nq4
            zq = NFREE // nq4
            for cch in range(nq4):
                za, zb = Z0 + cch * zq, Z0 + (cch + 1) * zq
                nc.sync.dma_start(allin[:, za:zb], allin_d[:, za:zb])
                a, bnd = cch * dq, min(DWIN, (cch + 1) * dq)
                nc.sync.dma_start(allin[:, a:bnd], allin_d[:, a:bnd])

            def seg(off, size):
                return allin[:, off : off + size]

            # Software pipeline, one tile look-ahead:
            #  - tile u+1's affinity (center matmul, ACT abs/exp) is issued
            #    mid-tile-u, so tile u's broadcasts never wait on it;
            #  - the three Pool-multiplied taps of tile u+1 (broadcast + ACT
            #    copy + Pool mult) are issued during tile u, so Pool's long
            #    serial chain has a full period of slack.
            POOL_KS = (1, 3, 6)     # pool-multiplied taps (pipelined ahead)
            SBUF_KS = (8,)          # ACT copy + DVE 2x-mode multiply
            # DIRECT_KS multiply straight from PSUM on the DVE

            def mk_aff(un):
                basez = un * NTILE
                zc = zcps.tile([P, NTILE], f32, tag="zc")
                nc.tensor.matmul(
                    zc[:], seg(M0 + 17 * P, P), seg(Z0 + basez, NTILE),
                    start=True, stop=True, skip_group_check=True,
                )
                afft = apool.tile([P, NTILE], f16, tag="afft")
                nc.scalar.activation(afft[:], zc[:], AF.Abs, scale=-ALPHA)
                nc.scalar.activation(afft[:], afft[:], AF.Exp, scale=-1.0)
                return afft

            def bcast(k, afft):
                jj = NC_KS.index(k)
                ap2 = affps.tile([P, NTILE], f32, tag="ap2")
                nc.tensor.matmul(
                    ap2[:], seg(M0 + (9 + jj) * P, P), afft[:],
                    start=True, stop=True, skip_group_check=True,
                )
                return ap2

            def pool_tap(k, afft, un):
                """broadcast + ACT copy + Pool multiply for tile un's tap k."""
                ap2 = bcast(k, afft)
                ap_sb = aspool.tile([P, NTILE], f16, tag="apsb")
                nc.scalar.copy(ap_sb[:], ap2[:])
                i, j = TAPS[k]
                v2 = vpool.tile([P, NTILE], f16, tag="v")
                nc.gpsimd.tensor_tensor(
                    v2[:], seg(D0 + un * NTILE + i * W + j, NTILE), ap_sb[:],
                    AluOpType.mult,
                )
                return v2

            def dve_tap(k, afft, un):
                """broadcast + DVE multiply (direct f32 PSUM, or via ACT f16
                copy for the 2x-mode tap)."""
                ap2 = bcast(k, afft)
                i, j = TAPS[k]
                v2 = vpool.tile([P, NTILE], f16, tag="v")
                if k in DIRECT_KS:
                    nc.vector.tensor_tensor(
                        v2[:], seg(D0 + un * NTILE + i * W + j, NTILE),
                        ap2[:], AluOpType.mult,
                    )
                else:
                    ap_sb = aspool.tile([P, NTILE], f16, tag="apsb")
                    nc.scalar.copy(ap_sb[:], ap2[:])
                    nc.vector.tensor_tensor(
                        v2[:], seg(D0 + un * NTILE + i * W + j, NTILE),
                        ap_sb[:], AluOpType.mult,
                    )
                return v2

            # prologue: tile 0 affinity + tile 0's pool taps
            affts = {0: mk_aff(0)}
            pool_v2 = {0: {k: pool_tap(k, affts[0], 0) for k in POOL_KS}}

            for u in range(NT):
                base = u * NTILE
                afft = affts.pop(u)
                outp = outps.tile([P, NTILE], f32, tag="outp")

                def out_mm(k, rhs, start=False, stop=False):
                    nc.tensor.matmul(
                        outp[:], seg(M0 + k * P, P), rhs,
                        start=start, stop=stop, skip_group_check=True,
                    )

                if u + 1 < NT:
                    # center matmul for u+1 heads the PE queue; its abs/exp
                    # are emitted later so tap copies head the ACT queue
                    basez = (u + 1) * NTILE
                    zc = zcps.tile([P, NTILE], f32, tag="zc")
                    nc.tensor.matmul(
                        zc[:], seg(M0 + 17 * P, P), seg(Z0 + basez, NTILE),
                        start=True, stop=True, skip_group_check=True,
                    )

                # this tile's DVE taps: broadcasts + multiplies
                v2s = {k: dve_tap(k, afft, u) for k in SBUF_KS + DIRECT_KS}
                # center tap opens the accumulation (needs no multiply)
                out_mm(4, seg(D0 + base + 1 * W + 1, NTILE), start=True)

                if u + 1 < NT:
                    # affinity abs/exp for u+1, then u+1's pool taps
                    afft_n = apool.tile([P, NTILE], f16, tag="afft")
                    nc.scalar.activation(afft_n[:], zc[:], AF.Abs, scale=-ALPHA)
                    nc.scalar.activation(afft_n[:], afft_n[:], AF.Exp, scale=-1.0)
                    affts[u + 1] = afft_n
                    pool_v2[u + 1] = {
                        k: pool_tap(k, afft_n, u + 1) for k in POOL_KS
                    }

                # drain the output matmuls: interleave ready-early pool taps
                # (computed last tile) with the DVE taps in arrival order
                pv = pool_v2.pop(u)
                order = [8, 1, 0, 3, 2, 6, 5, 7]
                for ki, k in enumerate(order):
                    v2 = pv[k] if k in POOL_KS else v2s[k]
                    out_mm(k, v2[:], stop=(ki == len(order) - 1))

                # DVE copy PSUM -> packed f16 SBUF (2 rows x 254 contiguous),
                # then one strided DMA (partitions (q,o) -> dst dims)
                osb = ospool.tile([P, 2 * OW], f16, tag="osb")
                nc.vector.tensor_copy(
                    osb[:],
                    outp[:].rearrange("p (x y) -> p x y", y=W)[:, :, 0:OW],
                )
                x0 = 2 * u
                nq_full = NQ if x0 + 2 <= 30 else NQ - 1
                dst_ap = AP(
                    out_d[:].tensor,
                    x0 * OW,
                    [[QROWS * OW, nq_full], [OH * OW, O], [1, 2 * OW]],
                )
                nc.sync.dma_start(dst_ap, osb[0 : 16 * nq_full, :])
                if nq_full < NQ:
                    # strip 7 has only 30 valid output rows (224..253)
                    nrows = max(0, min(x0 + 2, OH - QROWS * 7) - x0)
                    if nrows:
                        nc.sync.dma_start(
                            out_d[:, QROWS * 7 + x0 : QROWS * 7 + x0 + nrows, :],
                            osb[16 * 7 : 16 * 7 + 16, 0 : nrows * OW],
                        )
    nc.compile()
    return nc


def _pack_inputs(data, depth, weight):
    """Build the [B, 128, TOT] packed input: data windows, shifted depth
    taps, and the weight/selection matrices."""
    HP = H + 3
    data_p = np.zeros((B, C, HP * W), np.float32)
    data_p[:, :, : H * W] = data.reshape(B, C, H * W)
    depth_p = np.zeros((B, HP * W), np.float32)
    depth_p[:, : H * W] = depth.reshape(B, H * W)

    wsmat = np.zeros((NBLK, P, P), np.float32)
    for k in range(9):
        i, j = TAPS[k]
        blk = weight[:, :, i, j].T  # [c, o]
        for q in range(NQ):
            wsmat[k, 16 * q : 16 * q + 16, 16 * q : 16 * q + 16] = blk
    for jj, k in enumerate(NC_KS):
        for q in range(NQ):
            wsmat[9 + jj, 16 * q + k, 16 * q : 16 * q + 16] = 1.0
    wsmat[17] = np.eye(P, dtype=np.float32)
    for q in range(NQ):
        wsmat[17, 16 * q + 4, 16 * q : 16 * q + 16] -= 1.0
    wsmat_flat = wsmat.transpose(1, 0, 2).reshape(P, NBLK * P)

    allin = np.zeros((B, P, TOT), np.float16)
    for q in range(NQ):
        for c in range(C):
            p = 16 * q + c
            s = 32 * q * W
            allin[:, p, D0 : D0 + DWIN] = data_p[:, c, s : s + DWIN]
        for k, (i, j) in enumerate(TAPS):
            p = 16 * q + k
            s = (32 * q + i) * W + j
            allin[:, p, Z0 : Z0 + NFREE] = depth_p[:, s : s + NFREE]
    allin[:, :, M0:] = wsmat_flat[None]
    return allin


def run(inputs, **spmd_kwargs):
    from concourse.bass_utils import run_bass_kernel_spmd

    data = np.asarray(inputs["data"], np.float32)
    depth = np.asarray(inputs["depth"], np.float32)
    weight = np.asarray(inputs["weight"], np.float32)
    allin = _pack_inputs(data, depth, weight)

    if "nc" not in _CACHE:
        _CACHE["nc"] = _build_nc()
    nc = _CACHE["nc"]

    in_maps = [{"allin": np.ascontiguousarray(allin[b])} for b in range(B)]
    res = run_bass_kernel_spmd(nc, in_maps, core_ids=list(range(B)), **spmd_kwargs)
    out = np.stack([res.results[b]["out"] for b in range(B)]).astype(np.float32)
    return np.ascontiguousarray(out), res


def kernel(**inputs):
    out, _ = run(inputs)
    return out
